# revision 13
# baseline (speedup 1.0000x reference)
"""BoeNet greedy BFS rollout — Trainium2 Bass kernel (8 NeuronCores).

Strategy (v2): fully data-parallel, no collectives.
  Each core takes 512 of the 4096 flattened positions.
  Host prep: embedding rows pre-gathered + transposed (pure layout),
  Wout converted to bf16 once (replicated to all cores).
  Phase A (f32r): h0 = embT@Wp+bp -> 3-level greedy tree rollout ->
  masked mean pool, kept transposed as bf16 [H, pos] tiles (= lhsT for
  phase B). Gate sigmoid(z)>0.5 computed as z > -c_d exactly.
  Aggregation: root+level1 contributions via DVE on materialized
  children; level-2 contributions via PE (Wcs = WcL+WcR) into PSUM.
  Phase B (bf16): logits[pos, :] = pooled @ Wout for the core's own
  positions x full vocab; Wout streamed from HBM in 8 vocab blocks of
  4000 cols (double-buffered), output written bf16, upcast on host.
  No AllGather: phase A feeds phase B directly (zero stall).
"""
import sys

for _p in ('/opt/trn_rl_repo', '/opt/pypackages'):
    if _p not in sys.path:
        sys.path.insert(0, _p)

import numpy as np

B, S, V, E, H = 8, 512, 32000, 512, 512
NPOS = B * S              # 4096 flattened positions
NCORES = 8
PC_POS = NPOS // NCORES   # 512 positions per core
VB = 16                   # vocab blocks per core
VBW = V // VB             # 2000 columns per block
VCW = 500                 # columns per psum tile
NVC = VBW // VCW          # 4 psum tiles per block
MAX_DEPTH = 3
DEPTH_EMBED_SCALE = 0.01
SIB_SCALE = 1.0 / np.sqrt(H)

_CACHE = {}


def _build():
    import concourse.bass as bass
    import concourse.bacc as bacc
    import concourse.tile as tile
    import concourse.mybir as mybir
    from contextlib import ExitStack

    F32 = mybir.dt.float32
    F32R = mybir.dt.float32r
    BF16 = mybir.dt.bfloat16
    AF = mybir.ActivationFunctionType
    OP = mybir.AluOpType

    nc = bacc.Bacc("TRN2", target_bir_lowering=False, debug=False,
                   num_devices=NCORES)

    hembt_d = nc.dram_tensor("hembt", [E, PC_POS], F32, kind="ExternalInput")
    wp_d = nc.dram_tensor("wp", [E, H], F32, kind="ExternalInput")
    wc_d = nc.dram_tensor("wc", [H, 2 * H], F32, kind="ExternalInput")
    wcs_d = nc.dram_tensor("wcs", [H, H], F32, kind="ExternalInput")
    wg_d = nc.dram_tensor("wg", [H, 1], F32, kind="ExternalInput")
    rows_d = nc.dram_tensor("rows", [5, H], F32, kind="ExternalInput")
    cols_d = nc.dram_tensor("cols", [128, 12], F32, kind="ExternalInput")
    thr_d = nc.dram_tensor("thr", [1, 4], F32, kind="ExternalInput")
    iden_d = nc.dram_tensor("iden", [128, 128], F32, kind="ExternalInput")
    wout_d = nc.dram_tensor("wout", [H, V], BF16, kind="ExternalInput")
    logits_d = nc.dram_tensor("logits", [PC_POS, V], BF16,
                              kind="ExternalOutput")

    R_BS = 3  # rows_d row index of biasL+biasR

    def cp(out_ap, in_ap):
        nc.scalar.activation(out_ap, in_ap, AF.Copy)

    with tile.TileContext(nc) as tc, ExitStack() as ctx:
        const = ctx.enter_context(tc.tile_pool(name="const", bufs=1))
        wres = ctx.enter_context(tc.tile_pool(name="wres", bufs=1))
        popool = ctx.enter_context(tc.tile_pool(name="popool", bufs=1))

        rows_sb = const.tile([1, 5 * H], F32R, tag="rows")
        ones_row = rows_sb[0:1, 4 * H:5 * H]
        thr_sb = const.tile([1, 4], F32, tag="thr")
        cols_sb = const.tile([128, 12], F32, tag="cols")
        wg_sb = const.tile([128, 4], F32R, tag="wg")
        identity = const.tile([128, 128], F32R, tag="ident")

        def load_consts():
            nc.sync.dma_start(rows_sb[:],
                              rows_d[:].rearrange("a b -> () (a b)").bitcast(F32R))
            nc.sync.dma_start(thr_sb[:], thr_d[:])
            nc.sync.dma_start(cols_sb[:], cols_d[:])
            for hc in range(4):
                nc.sync.dma_start(wg_sb[:, hc:hc + 1],
                                  wg_d[hc * 128:(hc + 1) * 128, :].bitcast(F32R))
            nc.sync.dma_start(identity[:], iden_d[:].bitcast(F32R))

        # pooled lhsT tiles for phase B (bf16, persistent, pc-split so
        # each LDWEIGHTS reads a whole tile from offset 0)
        po = [[popool.tile([128, 128], BF16, tag=f"po{jc}_{pc}",
                           name=f"po{jc}_{pc}") for pc in range(4)]
              for jc in range(4)]

        # ---------------- Phase A ----------------
        with ExitStack() as actx:
            npool = actx.enter_context(tc.tile_pool(name="npool", bufs=1))
            wcpool = actx.enter_context(tc.tile_pool(name="wcpool", bufs=1))
            chpool = actx.enter_context(tc.tile_pool(name="chpool", bufs=1))
            mpool = actx.enter_context(tc.tile_pool(name="mpool", bufs=4))
            ebpool = actx.enter_context(tc.tile_pool(name="ebpool", bufs=3))
            rpool = actx.enter_context(tc.tile_pool(name="rpool", bufs=1))
            scr = actx.enter_context(tc.tile_pool(name="scr", bufs=4, space="PSUM"))
            aggp = actx.enter_context(tc.tile_pool(name="aggp", bufs=4, space="PSUM"))

            # phase-A inputs (DMA priority order: earliest-needed first)
            hembT = []
            for ec in range(4):
                t = npool.tile([128, PC_POS], F32R, tag=f"he{ec}", name=f"he{ec}")
                nc.sync.dma_start(t[:], hembt_d[ec * 128:(ec + 1) * 128, :].bitcast(F32R))
                hembT.append(t)
            wp_sb = []
            for ec in range(4):
                t = npool.tile([128, 512], F32R, tag=f"wp{ec}", name=f"wp{ec}")
                nc.sync.dma_start(t[:], wp_d[ec * 128:(ec + 1) * 128, :].bitcast(F32R))
                wp_sb.append(t)
            load_consts()
            wc_sb = []
            for hc in range(4):
                t = wcpool.tile([128, 1024], F32R, tag=f"wc{hc}", name=f"wc{hc}")
                nc.sync.dma_start(t[:], wc_d[hc * 128:(hc + 1) * 128, :].bitcast(F32R))
                wc_sb.append(t)
            wcs_sb = []
            for hc in range(4):
                t = npool.tile([128, 512], F32R, tag=f"wcs{hc}", name=f"wcs{hc}")
                nc.sync.dma_start(t[:], wcs_d[hc * 128:(hc + 1) * 128, :].bitcast(F32R))
                wcs_sb.append(t)

            # phase-B streamed weights: first two vocab blocks prefetch now
            # (drain during phase A); rest stream via the ring in phase B.
            wo_tiles = {}

            def wo_load(vb):
                tl = []
                for hc in range(4):
                    t = wres.tile([128, VBW], BF16, tag=f"wo{hc}",
                                  name=f"wo{hc}_{vb}", bufs=2)
                    nc.sync.dma_start(
                        t[:], wout_d[hc * 128:(hc + 1) * 128,
                                     vb * VBW:(vb + 1) * VBW])
                    tl.append(t)
                wo_tiles[vb] = tl

            wo_load(0)
            wo_load(1)

            # h0 = embT@Wp + bp  (bp folded into psum->sbuf copy bias)
            h0_sb = []
            for hc in range(4):
                ps = scr.tile([128, 512], F32, tag="s", name="h0ps")
                for ec in range(4):
                    nc.tensor.matmul(ps[:], wp_sb[ec][:, hc * 128:(hc + 1) * 128],
                                     hembT[ec][:], start=(ec == 0), stop=(ec == 3))
                t = npool.tile([128, 512], F32R, tag=f"h0_{hc}", name=f"h0_{hc}")
                nc.scalar.activation(t[:], ps[:], AF.Identity,
                                     bias=cols_sb[:, hc:hc + 1])
                h0_sb.append(t)

            # agg accumulator in PSUM, initialized with the root (h0) term
            # via identity matmul (h0_sb already includes bp)
            agg_ps = []
            for jc in range(4):
                ap_ = aggp.tile([128, 512], F32, tag="agg", name=f"agg{jc}")
                nc.tensor.matmul(ap_[:], identity[:], h0_sb[jc][:],
                                 start=True, stop=False, skip_group_check=True)
                agg_ps.append(ap_)

            def gate(node, depth, parent_e):
                zp = scr.tile([1, 512], F32, tag="s", name="zp")
                for hc in range(4):
                    nc.tensor.matmul(zp[:], wg_sb[:, hc:hc + 1], node[hc][:],
                                     start=(hc == 0), stop=(hc == 3))
                e = rpool.tile([1, 512], F32R, tag="erow", name="erow", bufs=7)
                nc.vector.tensor_scalar(e[:], zp[:], thr_sb[0:1, depth:depth + 1],
                                        None, OP.is_gt)
                if parent_e is not None:
                    nc.vector.tensor_mul(e[:], e[:], parent_e[:])
                return e

            def ebroadcast(e_row):
                ebp = scr.tile([128, 512], F32, tag="s", name="ebp")
                nc.tensor.matmul(ebp[:], ones_row[0:1, 0:128], e_row[:],
                                 start=True, stop=True)
                eb = ebpool.tile([128, 512], F32R, tag="eb", name="eb")
                cp(eb[:], ebp[:])
                return eb

            def agg_contrib_pe(node, eb):
                # agg += Wcs^T (eb * node)   (level-3 children, unmaterialized)
                mn = []
                for hc in range(4):
                    m = mpool.tile([128, 512], F32R, tag="mn", name=f"mn{hc}")
                    nc.vector.tensor_mul(m[:], node[hc][:], eb[:])
                    mn.append(m)
                for jc in range(4):
                    for hc in range(4):
                        nc.tensor.matmul(agg_ps[jc][:],
                                         wcs_sb[hc][:, jc * 128:(jc + 1) * 128],
                                         mn[hc][:], start=False, stop=False,
                                         skip_group_check=True)

            def children(node, lvl, nbufs):
                out = []
                for side in (0, 1):
                    child = []
                    for jc2 in range(4):
                        jq = side * 4 + jc2
                        ps = scr.tile([128, 512], F32, tag="s", name="chps")
                        for hc in range(4):
                            nc.tensor.matmul(ps[:], wc_sb[hc][:, jq * 128:(jq + 1) * 128],
                                             node[hc][:], start=(hc == 0), stop=(hc == 3))
                        t = chpool.tile([128, 512], F32R, tag=f"ch{lvl}",
                                        name=f"ch{lvl}_{side}_{jc2}", bufs=nbufs)
                        nc.scalar.activation(
                            t[:], ps[:], AF.Identity,
                            bias=cols_sb[:, 4 + side * 4 + jc2: 5 + side * 4 + jc2])
                        child.append(t)
                    out.append(child)
                return out

            with nc.allow_low_precision(reason="f32r matmul inputs"):
                # level-1 children first (pure PE work), gates/DVE behind
                n10, n11 = children(h0_sb, 1, 8)
                e0 = gate(h0_sb, 0, None)
                agg_contrib_pe(h0_sb, ebroadcast(e0))

                # n10 subtree fully (children, gates, contribs) before n11's
                # children are created, so the ch2/psum rings never wait on
                # not-yet-issued consumers (deadlock avoidance).
                n20, n21 = children(n10, 2, 10)
                e10 = gate(n10, 1, e0)
                agg_contrib_pe(n10, ebroadcast(e10))
                e20 = gate(n20, 2, e10)
                agg_contrib_pe(n20, ebroadcast(e20))
                e21 = gate(n21, 2, e10)
                agg_contrib_pe(n21, ebroadcast(e21))

                n22, n23 = children(n11, 2, 10)
                e11 = gate(n11, 1, e0)
                agg_contrib_pe(n11, ebroadcast(e11))
                e22 = gate(n22, 2, e11)
                agg_contrib_pe(n22, ebroadcast(e22))
                e23 = gate(n23, 2, e11)
                agg_contrib_pe(n23, ebroadcast(e23))

                # esum over all 7 nodes; bias term enters via bsum (outer) esum
                esum = rpool.tile([1, 512], F32R, tag="esum", name="esum", bufs=1)
                nc.vector.tensor_add(esum[:], e0[:], e10[:])
                nc.vector.tensor_add(esum[:], esum[:], e11[:])
                nc.vector.tensor_add(esum[:], esum[:], e20[:])
                nc.vector.tensor_add(esum[:], esum[:], e21[:])
                nc.vector.tensor_add(esum[:], esum[:], e22[:])
                nc.vector.tensor_add(esum[:], esum[:], e23[:])
                for jc in range(4):
                    nc.tensor.matmul(agg_ps[jc][:],
                                     rows_sb[0:1, R_BS * H + jc * 128: R_BS * H + (jc + 1) * 128],
                                     esum[:], start=False, stop=True,
                                     skip_group_check=True)
                cnt = rpool.tile([1, 512], F32, tag="cnt", name="cnt", bufs=1)
                nc.vector.tensor_scalar(cnt[:], esum[:], 2.0, 1.0, OP.mult, OP.add)
                nc.vector.reciprocal(cnt[:], cnt[:])
                recipr = rpool.tile([1, 512], F32R, tag="recipr", name="recipr", bufs=1)
                nc.vector.tensor_copy(recipr[:], cnt[:])
                rbp = scr.tile([128, 512], F32, tag="s", name="rbp")
                nc.tensor.matmul(rbp[:], ones_row[0:1, 0:128], recipr[:],
                                 start=True, stop=True)
                rb = ebpool.tile([128, 512], F32, tag="rb", name="rb", bufs=1)
                cp(rb[:], rbp[:])
                # pooled = agg_ps * (1/cnt), to bf16 lhsT tiles (pc-split,
                # pc-major so phase B's first group unblocks earliest)
                for pc in range(4):
                    for jc in range(4):
                        nc.vector.tensor_mul(
                            po[jc][pc][:], agg_ps[jc][:, pc * 128:(pc + 1) * 128],
                            rb[:, pc * 128:(pc + 1) * 128])

        # ---------------- Phase B ----------------
        with ExitStack() as bctx, \
                nc.allow_low_precision(reason="bf16 matmul inputs"):
            stp = bctx.enter_context(tc.tile_pool(name="stp", bufs=2))
            mmp = bctx.enter_context(tc.tile_pool(name="mmp", bufs=8, space="PSUM"))

            for vb in range(VB):
                for pc in range(4):
                    pst = [mmp.tile([128, VCW], F32, tag="mm", name=f"mm{v}",
                                    bufs=8) for v in range(NVC)]
                    for hc in range(4):
                        for v in range(NVC):
                            nc.tensor.matmul(
                                pst[v][:],
                                po[hc][pc][:],
                                wo_tiles[vb][hc][:, v * VCW:(v + 1) * VCW],
                                start=(hc == 0), stop=(hc == 3))
                    stage = stp.tile([128, VBW], BF16, tag="stage", name="stage")
                    for v in range(NVC):
                        dst = stage[:, v * VCW:(v + 1) * VCW]
                        if v % 2 == 0:
                            nc.vector.tensor_copy(dst, pst[v][:])
                        else:
                            cp(dst, pst[v][:])
                        if v == NVC // 2 - 1:
                            nc.sync.dma_start(
                                logits_d[pc * 128:(pc + 1) * 128,
                                         vb * VBW:vb * VBW + 2 * VCW],
                                stage[:, 0:2 * VCW])
                    nc.sync.dma_start(
                        logits_d[pc * 128:(pc + 1) * 128,
                                 vb * VBW + 2 * VCW:(vb + 1) * VBW],
                        stage[:, 2 * VCW:])
                # stream in the block after next (ring bufs=2)
                if vb + 2 < VB:
                    wo_load(vb + 2)

    nc.compile()
    return nc


def _get_nc():
    if "nc" not in _CACHE:
        _CACHE["nc"] = _build()
    return _CACHE["nc"]


def _prep_inputs(tokens, emb, Wp, bp, Wc, bc, Wg, bg, dep, sib, Wout, bout):
    import ml_dtypes
    tokens = np.asarray(tokens).astype(np.int64).reshape(-1)
    emb = np.asarray(emb, dtype=np.float32)
    Wp = np.ascontiguousarray(np.asarray(Wp, dtype=np.float32))
    bp = np.asarray(bp, dtype=np.float32).reshape(-1)
    Wc = np.asarray(Wc, dtype=np.float32)
    bc = np.asarray(bc, dtype=np.float32).reshape(-1)
    Wg = np.ascontiguousarray(np.asarray(Wg, dtype=np.float32))
    bg = np.asarray(bg, dtype=np.float32).reshape(-1)
    dep = np.asarray(dep, dtype=np.float32)
    sib = np.asarray(sib, dtype=np.float32)
    Wout = np.asarray(Wout, dtype=np.float32)
    bout = np.asarray(bout, dtype=np.float32).reshape(-1)
    _CACHE["bout"] = bout.copy()

    wcs = np.ascontiguousarray(Wc[:, :H] + Wc[:, H:])
    biasL = bc[:H] + SIB_SCALE * sib[0]
    biasR = bc[H:] + SIB_SCALE * sib[1]
    rows = np.ascontiguousarray(
        np.stack([bp, biasL, biasR, biasL + biasR, np.ones(H, np.float32)]))
    cols = np.ascontiguousarray(np.concatenate(
        [bp.reshape(4, 128).T, biasL.reshape(4, 128).T, biasR.reshape(4, 128).T],
        axis=1).astype(np.float32))
    g = DEPTH_EMBED_SCALE * (dep[:MAX_DEPTH] @ Wg[:, 0]) + bg[0]
    thr = np.zeros((1, 4), np.float32)
    thr[0, :MAX_DEPTH] = -g

    wout_bf = np.ascontiguousarray(Wout.astype(ml_dtypes.bfloat16))
    iden = np.eye(128, dtype=np.float32)
    wc_c = np.ascontiguousarray(Wc)

    in_maps = []
    for c in range(NCORES):
        tk = tokens[c * PC_POS:(c + 1) * PC_POS]
        hembt = np.ascontiguousarray(emb[tk].T)  # [E, PC_POS] f32
        in_maps.append({
            "hembt": hembt, "wp": Wp, "wc": wc_c, "wcs": wcs, "wg": Wg,
            "rows": rows, "cols": cols, "thr": thr, "iden": iden,
            "wout": wout_bf,
        })
    return in_maps


def _post(res) -> np.ndarray:
    parts = [np.asarray(res.results[c]["logits"]) for c in range(NCORES)]
    logits = np.concatenate(parts, axis=0).astype(np.float32)
    bout = _CACHE.get("bout")
    if bout is not None and np.any(bout):
        logits += bout
    return logits.reshape(B, S, V)


def _enable_ldw_opt_once():
    # Flip walrus's --enable-ldw-opt for compiles issued from this process
    # (dedups back-to-back identical LDWEIGHTS; measured win, verified exact).
    import os
    if os.environ.get("NO_LDW_OPT"):
        return
    if _CACHE.get("ldw_patched"):
        return
    import concourse.bass_utils as bu
    _orig = bu.run_command

    def _patched(cmd, **kw):
        if isinstance(cmd, list):
            cmd = ["--enable-ldw-opt=true" if c == "--enable-ldw-opt=false" else c
                   for c in cmd]
        return _orig(cmd, **kw)

    bu.run_command = _patched
    _CACHE["ldw_patched"] = True


def kernel(**inputs) -> np.ndarray:
    from concourse.bass_utils import run_bass_kernel_spmd
    _enable_ldw_opt_once()
    nc = _get_nc()
    in_maps = _prep_inputs(**inputs)
    res = run_bass_kernel_spmd(nc, in_maps, list(range(NCORES)))
    return _post(res)


# revision 14
# speedup vs baseline: 1.2001x; 1.2001x over previous
"""BoeNet greedy BFS rollout — Trainium2 Bass kernel (8 NeuronCores).

Strategy (v2): fully data-parallel, no collectives.
  Each core takes 512 of the 4096 flattened positions.
  Host prep: embedding rows pre-gathered + transposed (pure layout),
  Wout converted to bf16 once (replicated to all cores).
  Phase A (f32r): h0 = embT@Wp+bp -> 3-level greedy tree rollout ->
  masked mean pool, kept transposed as bf16 [H, pos] tiles (= lhsT for
  phase B). Gate sigmoid(z)>0.5 computed as z > -c_d exactly.
  Aggregation: root+level1 contributions via DVE on materialized
  children; level-2 contributions via PE (Wcs = WcL+WcR) into PSUM.
  Phase B (bf16): logits[pos, :] = pooled @ Wout for the core's own
  positions x full vocab; Wout streamed from HBM in 8 vocab blocks of
  4000 cols (double-buffered), output written bf16, upcast on host.
  No AllGather: phase A feeds phase B directly (zero stall).
"""
import sys

for _p in ('/opt/trn_rl_repo', '/opt/pypackages'):
    if _p not in sys.path:
        sys.path.insert(0, _p)

import numpy as np

B, S, V, E, H = 8, 512, 32000, 512, 512
NPOS = B * S              # 4096 flattened positions
NCORES = 8
PC_POS = NPOS // NCORES   # 512 positions per core
VB = 16                   # vocab blocks per core
VBW = V // VB             # 2000 columns per block
VCW = 500                 # columns per psum tile
NVC = VBW // VCW          # 4 psum tiles per block
MAX_DEPTH = 3
DEPTH_EMBED_SCALE = 0.01
SIB_SCALE = 1.0 / np.sqrt(H)

_CACHE = {}


def _build():
    import concourse.bass as bass
    import concourse.bacc as bacc
    import concourse.tile as tile
    import concourse.mybir as mybir
    from contextlib import ExitStack

    F32 = mybir.dt.float32
    F32R = mybir.dt.float32r
    BF16 = mybir.dt.bfloat16
    AF = mybir.ActivationFunctionType
    OP = mybir.AluOpType

    nc = bacc.Bacc("TRN2", target_bir_lowering=False, debug=False,
                   num_devices=NCORES)

    hembt_d = nc.dram_tensor("hembt", [E, PC_POS], F32, kind="ExternalInput")
    wp_d = nc.dram_tensor("wp", [E, H], F32, kind="ExternalInput")
    wc_d = nc.dram_tensor("wc", [H, 2 * H], F32, kind="ExternalInput")
    wcs_d = nc.dram_tensor("wcs", [H, H], F32, kind="ExternalInput")
    wg_d = nc.dram_tensor("wg", [H, 1], F32, kind="ExternalInput")
    rows_d = nc.dram_tensor("rows", [5, H], F32, kind="ExternalInput")
    cols_d = nc.dram_tensor("cols", [128, 12], F32, kind="ExternalInput")
    thr_d = nc.dram_tensor("thr", [1, 4], F32, kind="ExternalInput")
    iden_d = nc.dram_tensor("iden", [128, 128], F32, kind="ExternalInput")
    wout_d = nc.dram_tensor("wout", [H, V], BF16, kind="ExternalInput")
    logits_d = nc.dram_tensor("logits", [PC_POS, V], BF16,
                              kind="ExternalOutput")

    R_BS = 3  # rows_d row index of biasL+biasR

    def cp(out_ap, in_ap):
        nc.scalar.activation(out_ap, in_ap, AF.Copy)

    with tile.TileContext(nc) as tc, ExitStack() as ctx:
        const = ctx.enter_context(tc.tile_pool(name="const", bufs=1))
        wres = ctx.enter_context(tc.tile_pool(name="wres", bufs=1))
        popool = ctx.enter_context(tc.tile_pool(name="popool", bufs=1))

        rows_sb = const.tile([1, 5 * H], F32R, tag="rows")
        ones_row = rows_sb[0:1, 4 * H:5 * H]
        thr_sb = const.tile([1, 4], F32, tag="thr")
        cols_sb = const.tile([128, 12], F32, tag="cols")
        wg_sb = const.tile([128, 4], F32R, tag="wg")
        identity = const.tile([128, 128], F32R, tag="ident")

        def load_consts():
            nc.sync.dma_start(rows_sb[:],
                              rows_d[:].rearrange("a b -> () (a b)").bitcast(F32R))
            nc.sync.dma_start(thr_sb[:], thr_d[:])
            nc.sync.dma_start(cols_sb[:], cols_d[:])
            for hc in range(4):
                nc.sync.dma_start(wg_sb[:, hc:hc + 1],
                                  wg_d[hc * 128:(hc + 1) * 128, :].bitcast(F32R))
            nc.sync.dma_start(identity[:], iden_d[:].bitcast(F32R))

        # pooled lhsT tiles for phase B (bf16, persistent, pc-split so
        # each LDWEIGHTS reads a whole tile from offset 0)
        po = [[popool.tile([128, 128], BF16, tag=f"po{jc}_{pc}",
                           name=f"po{jc}_{pc}") for pc in range(4)]
              for jc in range(4)]

        # ---------------- Phase A ----------------
        with ExitStack() as actx:
            npool = actx.enter_context(tc.tile_pool(name="npool", bufs=1))
            wcpool = actx.enter_context(tc.tile_pool(name="wcpool", bufs=1))
            chpool = actx.enter_context(tc.tile_pool(name="chpool", bufs=1))
            mpool = actx.enter_context(tc.tile_pool(name="mpool", bufs=8))
            ebpool = actx.enter_context(tc.tile_pool(name="ebpool", bufs=3))
            rpool = actx.enter_context(tc.tile_pool(name="rpool", bufs=1))
            scr = actx.enter_context(tc.tile_pool(name="scr", bufs=4, space="PSUM"))
            aggp = actx.enter_context(tc.tile_pool(name="aggp", bufs=4, space="PSUM"))

            # phase-A inputs (DMA priority order: earliest-needed first)
            hembT = []
            for ec in range(4):
                t = npool.tile([128, PC_POS], F32R, tag=f"he{ec}", name=f"he{ec}")
                nc.sync.dma_start(t[:], hembt_d[ec * 128:(ec + 1) * 128, :].bitcast(F32R))
                hembT.append(t)
            wp_sb = []
            for ec in range(4):
                t = npool.tile([128, 512], F32R, tag=f"wp{ec}", name=f"wp{ec}")
                nc.sync.dma_start(t[:], wp_d[ec * 128:(ec + 1) * 128, :].bitcast(F32R))
                wp_sb.append(t)
            load_consts()
            wc_sb = []
            for hc in range(4):
                t = wcpool.tile([128, 1024], F32R, tag=f"wc{hc}", name=f"wc{hc}")
                nc.sync.dma_start(t[:], wc_d[hc * 128:(hc + 1) * 128, :].bitcast(F32R))
                wc_sb.append(t)
            wcs_sb = []
            for hc in range(4):
                t = npool.tile([128, 512], F32R, tag=f"wcs{hc}", name=f"wcs{hc}")
                nc.sync.dma_start(t[:], wcs_d[hc * 128:(hc + 1) * 128, :].bitcast(F32R))
                wcs_sb.append(t)

            # phase-B streamed weights: first two vocab blocks prefetch now
            # (drain during phase A); rest stream via the ring in phase B.
            wo_tiles = {}

            def wo_load(vb):
                tl = []
                for hc in range(4):
                    t = wres.tile([128, VBW], BF16, tag=f"wo{hc}",
                                  name=f"wo{hc}_{vb}", bufs=2)
                    nc.sync.dma_start(
                        t[:], wout_d[hc * 128:(hc + 1) * 128,
                                     vb * VBW:(vb + 1) * VBW])
                    tl.append(t)
                wo_tiles[vb] = tl

            wo_load(0)
            wo_load(1)

            # h0 = embT@Wp + bp  (bp folded into psum->sbuf copy bias)
            h0_sb = []
            for hc in range(4):
                ps = scr.tile([128, 512], F32, tag="s", name="h0ps")
                for ec in range(4):
                    nc.tensor.matmul(ps[:], wp_sb[ec][:, hc * 128:(hc + 1) * 128],
                                     hembT[ec][:], start=(ec == 0), stop=(ec == 3))
                t = npool.tile([128, 512], F32R, tag=f"h0_{hc}", name=f"h0_{hc}")
                nc.scalar.activation(t[:], ps[:], AF.Identity,
                                     bias=cols_sb[:, hc:hc + 1])
                h0_sb.append(t)

            # agg accumulator in PSUM, initialized with the root (h0) term
            # via identity matmul (h0_sb already includes bp)
            agg_ps = []
            for jc in range(4):
                ap_ = aggp.tile([128, 512], F32, tag="agg", name=f"agg{jc}")
                nc.tensor.matmul(ap_[:], identity[:], h0_sb[jc][:],
                                 start=True, stop=False, skip_group_check=True)
                agg_ps.append(ap_)

            def gate(node, depth, parent_e):
                zp = scr.tile([1, 512], F32, tag="s", name="zp")
                for hc in range(4):
                    nc.tensor.matmul(zp[:], wg_sb[:, hc:hc + 1], node[hc][:],
                                     start=(hc == 0), stop=(hc == 3))
                e = rpool.tile([1, 512], F32R, tag="erow", name="erow", bufs=7)
                nc.vector.tensor_scalar(e[:], zp[:], thr_sb[0:1, depth:depth + 1],
                                        None, OP.is_gt)
                if parent_e is not None:
                    nc.vector.tensor_mul(e[:], e[:], parent_e[:])
                return e

            def ebroadcast(e_row):
                ebp = scr.tile([128, 512], F32, tag="s", name="ebp")
                nc.tensor.matmul(ebp[:], ones_row[0:1, 0:128], e_row[:],
                                 start=True, stop=True)
                eb = ebpool.tile([128, 512], F32R, tag="eb", name="eb")
                cp(eb[:], ebp[:])
                return eb

            def agg_contrib_pe(node, eb):
                # agg += Wcs^T (eb * node)   (level-3 children, unmaterialized)
                mn = []
                for hc in range(4):
                    m = mpool.tile([128, 512], F32R, tag="mn", name=f"mn{hc}")
                    nc.vector.tensor_mul(m[:], node[hc][:], eb[:])
                    mn.append(m)
                for jc in range(4):
                    for hc in range(4):
                        nc.tensor.matmul(agg_ps[jc][:],
                                         wcs_sb[hc][:, jc * 128:(jc + 1) * 128],
                                         mn[hc][:], start=False, stop=False,
                                         skip_group_check=True)

            def children(node, lvl, nbufs):
                out = []
                for side in (0, 1):
                    child = []
                    for jc2 in range(4):
                        jq = side * 4 + jc2
                        ps = scr.tile([128, 512], F32, tag="s", name="chps")
                        for hc in range(4):
                            nc.tensor.matmul(ps[:], wc_sb[hc][:, jq * 128:(jq + 1) * 128],
                                             node[hc][:], start=(hc == 0), stop=(hc == 3))
                        t = chpool.tile([128, 512], F32R, tag=f"ch{lvl}",
                                        name=f"ch{lvl}_{side}_{jc2}", bufs=nbufs)
                        nc.scalar.activation(
                            t[:], ps[:], AF.Identity,
                            bias=cols_sb[:, 4 + side * 4 + jc2: 5 + side * 4 + jc2])
                        child.append(t)
                    out.append(child)
                return out

            with nc.allow_low_precision(reason="f32r matmul inputs"):
                # level-1 children first (pure PE work), gates/DVE behind
                n10, n11 = children(h0_sb, 1, 8)
                e0 = gate(h0_sb, 0, None)
                agg_contrib_pe(h0_sb, ebroadcast(e0))

                n20, n21 = children(n10, 2, 16)
                e10 = gate(n10, 1, e0)
                agg_contrib_pe(n10, ebroadcast(e10))
                e20 = gate(n20, 2, e10)
                e21 = gate(n21, 2, e10)

                n22, n23 = children(n11, 2, 16)
                e11 = gate(n11, 1, e0)
                agg_contrib_pe(n11, ebroadcast(e11))
                e22 = gate(n22, 2, e11)
                e23 = gate(n23, 2, e11)

                # esum/count/reciprocal chain issued now so the DVE work
                # overlaps the four level-2 contribution matmul bursts below
                esum = rpool.tile([1, 512], F32R, tag="esum", name="esum", bufs=1)
                nc.vector.tensor_add(esum[:], e0[:], e10[:])
                nc.vector.tensor_add(esum[:], esum[:], e11[:])
                nc.vector.tensor_add(esum[:], esum[:], e20[:])
                nc.vector.tensor_add(esum[:], esum[:], e21[:])
                nc.vector.tensor_add(esum[:], esum[:], e22[:])
                nc.vector.tensor_add(esum[:], esum[:], e23[:])
                cnt = rpool.tile([1, 512], F32, tag="cnt", name="cnt", bufs=1)
                nc.vector.tensor_scalar(cnt[:], esum[:], 2.0, 1.0, OP.mult, OP.add)
                nc.vector.reciprocal(cnt[:], cnt[:])
                recipr = rpool.tile([1, 512], F32R, tag="recipr", name="recipr", bufs=1)
                nc.vector.tensor_copy(recipr[:], cnt[:])
                rbp = scr.tile([128, 512], F32, tag="s", name="rbp")
                nc.tensor.matmul(rbp[:], ones_row[0:1, 0:128], recipr[:],
                                 start=True, stop=True)
                rb = ebpool.tile([128, 512], F32, tag="rb", name="rb", bufs=1)
                cp(rb[:], rbp[:])

                agg_contrib_pe(n20, ebroadcast(e20))
                agg_contrib_pe(n21, ebroadcast(e21))
                agg_contrib_pe(n22, ebroadcast(e22))
                agg_contrib_pe(n23, ebroadcast(e23))
                for jc in range(4):
                    nc.tensor.matmul(agg_ps[jc][:],
                                     rows_sb[0:1, R_BS * H + jc * 128: R_BS * H + (jc + 1) * 128],
                                     esum[:], start=False, stop=True,
                                     skip_group_check=True)
                # pooled = agg_ps * (1/cnt), to bf16 lhsT tiles (pc-split,
                # pc-major so phase B's first group unblocks earliest)
                for pc in range(4):
                    for jc in range(4):
                        nc.vector.tensor_mul(
                            po[jc][pc][:], agg_ps[jc][:, pc * 128:(pc + 1) * 128],
                            rb[:, pc * 128:(pc + 1) * 128])

        # ---------------- Phase B ----------------
        with ExitStack() as bctx, \
                nc.allow_low_precision(reason="bf16 matmul inputs"):
            stp = bctx.enter_context(tc.tile_pool(name="stp", bufs=2))
            mmp = bctx.enter_context(tc.tile_pool(name="mmp", bufs=8, space="PSUM"))

            for vb in range(VB):
                for pc in range(4):
                    pst = [mmp.tile([128, VCW], F32, tag="mm", name=f"mm{v}",
                                    bufs=8) for v in range(NVC)]
                    for hc in range(4):
                        for v in range(NVC):
                            nc.tensor.matmul(
                                pst[v][:],
                                po[hc][pc][:],
                                wo_tiles[vb][hc][:, v * VCW:(v + 1) * VCW],
                                start=(hc == 0), stop=(hc == 3))
                    stage = stp.tile([128, VBW], BF16, tag="stage", name="stage")
                    for v in range(NVC):
                        dst = stage[:, v * VCW:(v + 1) * VCW]
                        if v % 2 == 0:
                            nc.vector.tensor_copy(dst, pst[v][:])
                        else:
                            cp(dst, pst[v][:])
                    nc.scalar.dma_start(
                        logits_d[pc * 128:(pc + 1) * 128,
                                 vb * VBW:(vb + 1) * VBW],
                        stage[:])
                # stream in the block after next (ring bufs=2)
                if vb + 2 < VB:
                    wo_load(vb + 2)

    nc.compile()
    return nc


def _get_nc():
    if "nc" not in _CACHE:
        _CACHE["nc"] = _build()
    return _CACHE["nc"]


def _prep_inputs(tokens, emb, Wp, bp, Wc, bc, Wg, bg, dep, sib, Wout, bout):
    import ml_dtypes
    tokens = np.asarray(tokens).astype(np.int64).reshape(-1)
    emb = np.asarray(emb, dtype=np.float32)
    Wp = np.ascontiguousarray(np.asarray(Wp, dtype=np.float32))
    bp = np.asarray(bp, dtype=np.float32).reshape(-1)
    Wc = np.asarray(Wc, dtype=np.float32)
    bc = np.asarray(bc, dtype=np.float32).reshape(-1)
    Wg = np.ascontiguousarray(np.asarray(Wg, dtype=np.float32))
    bg = np.asarray(bg, dtype=np.float32).reshape(-1)
    dep = np.asarray(dep, dtype=np.float32)
    sib = np.asarray(sib, dtype=np.float32)
    Wout = np.asarray(Wout, dtype=np.float32)
    bout = np.asarray(bout, dtype=np.float32).reshape(-1)
    _CACHE["bout"] = bout.copy()

    wcs = np.ascontiguousarray(Wc[:, :H] + Wc[:, H:])
    biasL = bc[:H] + SIB_SCALE * sib[0]
    biasR = bc[H:] + SIB_SCALE * sib[1]
    rows = np.ascontiguousarray(
        np.stack([bp, biasL, biasR, biasL + biasR, np.ones(H, np.float32)]))
    cols = np.ascontiguousarray(np.concatenate(
        [bp.reshape(4, 128).T, biasL.reshape(4, 128).T, biasR.reshape(4, 128).T],
        axis=1).astype(np.float32))
    g = DEPTH_EMBED_SCALE * (dep[:MAX_DEPTH] @ Wg[:, 0]) + bg[0]
    thr = np.zeros((1, 4), np.float32)
    thr[0, :MAX_DEPTH] = -g

    wout_bf = np.ascontiguousarray(Wout.astype(ml_dtypes.bfloat16))
    iden = np.eye(128, dtype=np.float32)
    wc_c = np.ascontiguousarray(Wc)

    in_maps = []
    for c in range(NCORES):
        tk = tokens[c * PC_POS:(c + 1) * PC_POS]
        hembt = np.ascontiguousarray(emb[tk].T)  # [E, PC_POS] f32
        in_maps.append({
            "hembt": hembt, "wp": Wp, "wc": wc_c, "wcs": wcs, "wg": Wg,
            "rows": rows, "cols": cols, "thr": thr, "iden": iden,
            "wout": wout_bf,
        })
    return in_maps


def _post(res) -> np.ndarray:
    parts = [np.asarray(res.results[c]["logits"]) for c in range(NCORES)]
    logits = np.concatenate(parts, axis=0).astype(np.float32)
    bout = _CACHE.get("bout")
    if bout is not None and np.any(bout):
        logits += bout
    return logits.reshape(B, S, V)


def _enable_ldw_opt_once():
    # Flip walrus's --enable-ldw-opt for compiles issued from this process
    # (dedups back-to-back identical LDWEIGHTS; measured win, verified exact).
    import os
    if os.environ.get("NO_LDW_OPT"):
        return
    if _CACHE.get("ldw_patched"):
        return
    import concourse.bass_utils as bu
    _orig = bu.run_command

    def _patched(cmd, **kw):
        if isinstance(cmd, list):
            cmd = ["--enable-ldw-opt=true" if c == "--enable-ldw-opt=false" else c
                   for c in cmd]
        return _orig(cmd, **kw)

    bu.run_command = _patched
    _CACHE["ldw_patched"] = True


def kernel(**inputs) -> np.ndarray:
    from concourse.bass_utils import run_bass_kernel_spmd
    _enable_ldw_opt_once()
    nc = _get_nc()
    in_maps = _prep_inputs(**inputs)
    res = run_bass_kernel_spmd(nc, in_maps, list(range(NCORES)))
    return _post(res)
